# revision 14
# baseline (speedup 1.0000x reference)
"""AL2Loss2d Trainium2 kernel.

Reference computation:
  inputs [8, 64, 512, 512] f32, targets [8, 512, 512] int64 (values 0..18)
  - per-class sums of the 64-dim pixel features (segment_sum over 2M pixels)
  - per-class counts
  - centers = sums / max(counts, 1); pairwise cosine similarity of the 19
    centers; CosineEmbeddingLoss-style reduction to a scalar.

Strategy: data-parallel over batch. Each of the 8 NeuronCores reads one
batch element (64 MiB) and computes partial [19, 65] (sums | counts) via
one-hot accumulating matmuls on the TensorEngine:
  - host pre-permutes the shard to pixel-major layout [128, J, 65]
    (64 channels + a ones column, so counts come out of the same matmul)
  - device per tile: DMA -> fp32->fp16 convert (ScalarE) -> one-hot of
    the targets via iota+is_equal (VectorE) -> per-128-pixel-chunk matmul
    psum[19,65] += onehot[128,19].T @ x[128,65] (TensorE, fp16)
  - the tail tiles are small so little compute remains after the last
    DMA byte lands (the kernel is HBM-bandwidth-bound at ~358 GB/s/core)
The tiny 19x19 cosine loss runs on host on the 8 gathered partials.
"""

import sys

import numpy as np

if "/opt/trn_rl_repo" not in sys.path:
    sys.path.insert(0, "/opt/trn_rl_repo")

from concourse import bacc, bass, mybir, tile  # noqa: E402
from concourse.bass_utils import run_bass_kernel_spmd  # noqa: E402

K = 19
CH = 64
CW = CH + 1  # 64 channels + ones column
NCORES = 8
NPART = 128
EPS = 1e-8


def segments(jtot: int, g: int, tail: int):
    """Split [0, jtot) into tiles of g chunks with a tapered tail."""
    segs = []
    j = 0
    ntail = g // tail if tail else 0
    while j < jtot - ntail * tail:
        segs.append((j, g))
        j += g
    while j < jtot:
        segs.append((j, tail))
        j += tail
    assert sum(s[1] for s in segs) == jtot
    return segs


def build(jtot: int, g: int, tail: int = 0) -> bass.Bass:
    """Build the per-core Bass program (pixels = 128 * jtot)."""
    segs = segments(jtot, g, tail)
    nc = bacc.Bacc(target_bir_lowering=False, trn_type="TRN2")
    x_ext = nc.declare_dram_parameter(
        "x", [NPART, jtot, CH], mybir.dt.float32, isOutput=False
    )
    t_ext = nc.declare_dram_parameter("t", [NPART, jtot], mybir.dt.int8, isOutput=False)
    out_ext = nc.declare_dram_parameter("out", [K, CW], mybir.dt.float32, isOutput=True)

    with tile.TileContext(nc) as tc:
        with (
            tc.tile_pool(name="const", bufs=1) as cpool,
            tc.tile_pool(name="xin", bufs=4) as xpool,
            tc.tile_pool(name="xh", bufs=3) as xhpool,
            tc.tile_pool(name="oh", bufs=3) as ohpool,
            tc.tile_pool(name="red", bufs=2) as redpool,
            tc.tile_pool(name="acc", bufs=1, space=bass.MemorySpace.PSUM) as psumpool,
            tc.tile_pool(name="outp", bufs=1) as opool,
        ):
            # First two x tiles go out on the gpsimd (SWDGE) ring so data
            # starts streaming while the sync ring's preamble clears.
            xts = []
            for si, (j0, gg) in enumerate(segs[:2]):
                xt = xpool.tile([NPART, g, CH], mybir.dt.float32, tag="xt")
                nc.gpsimd.dma_start(xt[:, :gg, :], x_ext[:, j0 : j0 + gg, :])
                xts.append(xt)

            t_sb = cpool.tile([NPART, jtot], mybir.dt.int8)
            nc.gpsimd.dma_start(t_sb[:], t_ext[:])
            iota = cpool.tile([NPART, g, K], mybir.dt.int32)
            nc.gpsimd.iota(
                iota[:], pattern=[[0, g], [1, K]], base=0, channel_multiplier=0
            )
            cnt = cpool.tile([NPART, K], mybir.dt.float32)
            nc.vector.memset(cnt[:], 0.0)

            acc = psumpool.tile([K, CH], mybir.dt.float32)
            nmm = sum(s[1] for s in segs)
            mm = 0
            for si, (j0, gg) in enumerate(segs):
                if si < 2:
                    xt = xts[si]
                else:
                    xt = xpool.tile([NPART, g, CH], mybir.dt.float32, tag="xt")
                    nc.sync.dma_start(xt[:, :gg, :], x_ext[:, j0 : j0 + gg, :])
                xh = xhpool.tile([NPART, g, CH], mybir.dt.float16, tag="xh")
                nc.scalar.mul(xh[:, :gg, :], xt[:, :gg, :], 1.0)
                oh = ohpool.tile([NPART, g, K], mybir.dt.float16, tag="oh")
                tb = t_sb[:, j0 : j0 + gg].unsqueeze(2).broadcast_to([NPART, gg, K])
                nc.vector.tensor_tensor(
                    out=oh[:, :gg, :],
                    in0=tb,
                    in1=iota[:, :gg, :],
                    op=mybir.AluOpType.is_equal,
                )
                # per-partition class counts: cnt[p, k] += sum_g oh[p, g, k]
                red = redpool.tile([NPART, K], mybir.dt.float32, tag="red")
                nc.vector.tensor_reduce(
                    red[:],
                    oh[:, :gg, :].rearrange("p g k -> p k g"),
                    axis=mybir.AxisListType.X,
                    op=mybir.AluOpType.add,
                )
                nc.vector.tensor_add(cnt[:], cnt[:], red[:])
                for i in range(gg):
                    nc.tensor.matmul(
                        acc[:],
                        oh[:, i, :],
                        xh[:, i, :],
                        start=(mm == 0),
                        stop=(mm == nmm - 1),
                    )
                    mm += 1
            # counts: fold the [128, K] per-partition histogram with one matmul
            cnt16 = opool.tile([NPART, K], mybir.dt.float16)
            nc.vector.tensor_copy(cnt16[:], cnt[:])
            ones16 = opool.tile([NPART, 1], mybir.dt.float16)
            nc.vector.memset(ones16[:], 1.0)
            cacc = psumpool.tile([K, 1], mybir.dt.float32)
            nc.tensor.matmul(cacc[:], cnt16[:], ones16[:], start=True, stop=True)

            out_sb = opool.tile([K, CW], mybir.dt.float32)
            nc.vector.tensor_copy(out_sb[:, :CH], acc[:])
            nc.vector.tensor_copy(out_sb[:, CH : CH + 1], cacc[:])
            nc.sync.dma_start(out_ext[:], out_sb[:])
    nc.compile()
    return nc


def prep_shard(x_b: np.ndarray, t_b: np.ndarray, jtot: int):
    """x_b [64, H, W] f32, t_b [H, W] int -> device arrays.

    Pixel p*jtot + j lands at partition p, column j:
      xdev[p, j, 0:64] = features, tdev[p, j] = class id (int8)
    """
    xr = x_b.reshape(CH, NPART, jtot)
    xdev = np.ascontiguousarray(xr.transpose(1, 2, 0))
    tdev = np.ascontiguousarray(t_b.reshape(NPART, jtot).astype(np.int8))
    return xdev, tdev


_NC_CACHE: dict = {}
TRACE = False  # set True (e.g. from test.py) to profile; result lands here
LAST_RESULT = None
G = 64
TAIL = 16


def _get_nc(jtot: int) -> bass.Bass:
    key = (jtot, G, TAIL)
    if key not in _NC_CACHE:
        _NC_CACHE[key] = build(jtot, G, TAIL)
    return _NC_CACHE[key]


def finish(partials: np.ndarray) -> np.float32:
    """partials [ncores, K, CW] -> scalar loss (host, mirrors reference)."""
    total = partials.sum(axis=0, dtype=np.float64)
    sums = total[:, :CH]
    counts = total[:, CH]
    centers = sums / np.maximum(counts, 1.0)[:, None]
    norms = np.maximum(np.sqrt((centers * centers).sum(axis=1)), EPS)
    cn = centers / norms[:, None]
    S = cn @ cn.T
    eye = np.eye(K, dtype=bool)
    per_pair = np.where(eye, 1.0 - S, np.maximum(S, 0.0))
    return np.float32(per_pair.sum() / (K * K * K))


def kernel(inputs: np.ndarray, targets: np.ndarray) -> np.ndarray:
    B, C, H, W = inputs.shape
    assert (B, C) == (NCORES, CH)
    jtot = H * W // NPART
    nc = _get_nc(jtot)

    in_maps = []
    for i in range(NCORES):
        xdev, tdev = prep_shard(np.asarray(inputs[i]), np.asarray(targets[i]), jtot)
        in_maps.append({"x": xdev, "t": tdev})

    res = run_bass_kernel_spmd(
        nc, in_maps, core_ids=list(range(NCORES)), trace=TRACE
    )
    global LAST_RESULT
    LAST_RESULT = res
    partials = np.stack([r["out"] for r in res.results])
    return np.asarray(finish(partials))
